# revision 29
# baseline (speedup 1.0000x reference)
"""Trainium2 Bass kernel for ChannelCrissCrossAttention (v3).

Shapes (hardcoded): B=8, IN=128, C=16, V=T=64.
Sharding: pure data parallel, one batch element per NeuronCore (8 cores).

Math (same as v2): per batch element
  q,k,v = conv3x3(x; wq/wk/wv) -> [C, V, T].  Row grids r=(c,a):
  G_r[x,j] = exp(q[c,a,x]*k[c,a,j]) serve tt (natural) and vv (spatial
  transpose).  cc grids live on 16-elem strips of the flat channel-major
  layout at spatially-transposed positions.  Z = S_tt + S_vv + S_cc;
  outputs W_*/Z; stacked reverse conv; gamma*out + x.

v3 vs v2:
 - bf16 matmuls (4x PE throughput vs fp32) and bf16 DVE element-wise ops
   (2x_1p perf mode).
 - No DRAM scratch at all: every spread/rearrange is an SBUF->SBUF DMA.
 - S/W row-sums via fold-trees (bf16 tensor_add at 2x) + short f32 reduce.
 - Outer-product muls (the only 1x DVE pass) offloaded to the Pool/GpSimd
   engine so DVE, Pool, ACT and PE all run concurrently.
 - Batched combine (Z, 1/Z, three weighted outputs) after the b-loop.
"""

import sys

sys.path.insert(0, "/opt/trn_rl_repo")

import numpy as np
import ml_dtypes

import concourse.bass as bass
import concourse.tile as tile
from concourse import bacc, mybir
from concourse.bass_utils import run_bass_kernel_spmd

F32 = mybir.dt.float32
BF16 = mybir.dt.bfloat16
AF = mybir.ActivationFunctionType
ALU = mybir.AluOpType
AX = mybir.AxisListType

IN, C, V, T = 128, 16, 64, 64
CH3 = 3 * C  # 48
NPOS = V * T  # 4096
PW = 66  # padded spatial width


def _build_program(niter=1):
    nc = bacc.Bacc("TRN2", target_bir_lowering=False, debug=False)

    x_d = nc.dram_tensor("x", [IN, V, T], BF16, kind="ExternalInput")
    wqkv_d = nc.dram_tensor("wqkv", [IN, 9 * CH3], BF16, kind="ExternalInput")
    bqkv_d = nc.dram_tensor("bqkv", [CH3, 1], F32, kind="ExternalInput")
    wr_d = nc.dram_tensor("wr", [CH3, 9 * IN], BF16, kind="ExternalInput")
    gb_d = nc.dram_tensor("gb", [IN, 1], F32, kind="ExternalInput")
    id_d = nc.dram_tensor("ident", [128, 128], F32, kind="ExternalInput")
    out_d = nc.dram_tensor("out", [IN, V, T], F32, kind="ExternalOutput")
    qkv_h = nc.dram_tensor("qkv_h", [CH3, V, T], BF16)
    st_h = nc.dram_tensor("st_h", [2 * C, V, T], BF16)
    ocat_h = nc.dram_tensor("ocat_h", [CH3, V, T], BF16)

    with tile.TileContext(nc) as tc:
        if niter == 1:
            _body(nc, tc, x_d, wqkv_d, bqkv_d, wr_d, gb_d, id_d, out_d, qkv_h,
                  st_h, ocat_h)
        else:
            with tc.For_i(0, niter, 1):
                _body(nc, tc, x_d, wqkv_d, bqkv_d, wr_d, gb_d, id_d, out_d,
                      qkv_h, st_h, ocat_h)

    nc.compile()
    return nc


def _body(nc, tc, x_d, wqkv_d, bqkv_d, wr_d, gb_d, id_d, out_d, qkv_h, st_h,
          ocat_h):
    from contextlib import ExitStack
    ctx = ExitStack()
    persist = ctx.enter_context(tc.tile_pool(name="persist", bufs=1))
    pio = ctx.enter_context(tc.tile_pool(name="pio", bufs=2))
    pP = ctx.enter_context(tc.tile_pool(name="pP", bufs=2))
    pG = ctx.enter_context(tc.tile_pool(name="pG", bufs=2))
    pH = ctx.enter_context(tc.tile_pool(name="pH", bufs=1))
    pF = ctx.enter_context(tc.tile_pool(name="pF", bufs=1))
    pcomb = ctx.enter_context(tc.tile_pool(name="pcomb", bufs=1))
    psum1 = ctx.enter_context(
        tc.tile_pool(name="psum1", bufs=2, space=bass.MemorySpace.PSUM))
    psum2 = ctx.enter_context(
        tc.tile_pool(name="psum2", bufs=2, space=bass.MemorySpace.PSUM))
    psumS = ctx.enter_context(
        tc.tile_pool(name="psumS", bufs=1, space=bass.MemorySpace.PSUM))

    # ---- Phase 0: weights + padded input ----
    wqkv = persist.tile([IN, 9 * CH3], BF16)
    nc.sync.dma_start(wqkv[:], wqkv_d.ap())
    bqkv = persist.tile([CH3, 1], F32)
    nc.sync.dma_start(bqkv[:], bqkv_d.ap())
    wr = persist.tile([CH3, 9 * IN], BF16)
    nc.sync.dma_start(wr[:], wr_d.ap())
    gb = persist.tile([IN, 1], F32)
    nc.sync.dma_start(gb[:], gb_d.ap())
    ident = persist.tile([128, 128], F32)
    nc.sync.dma_start(ident[:], id_d.ap())

    xpad = persist.tile([IN, PW * PW], BF16)
    xpad_v = xpad[:].rearrange("p (v t) -> p v t", v=PW)
    # zero only the 1-wide border (top/bottom rows, left/right cols)
    nc.gpsimd.memset(xpad_v[:, 0, :], 0.0)
    nc.gpsimd.memset(xpad_v[:, PW - 1, :], 0.0)
    nc.gpsimd.memset(xpad_v[:, 1:PW - 1, 0:1], 0.0)
    nc.gpsimd.memset(xpad_v[:, 1:PW - 1, PW - 1:PW], 0.0)
    for m in range(8):
        nc.sync.dma_start(
            xpad_v[:, 1 + m * 8:1 + (m + 1) * 8, 1:1 + T],
            x_d.ap()[:, m * 8:(m + 1) * 8, :])

    opad = persist.tile([CH3, PW * PW], BF16)
    opad_v = opad[:].rearrange("p (v t) -> p v t", v=PW)
    nc.gpsimd.memset(opad_v[:, 0, :], 0.0)
    nc.gpsimd.memset(opad_v[:, PW - 1, :], 0.0)
    nc.gpsimd.memset(opad_v[:, 1:PW - 1, 0:1], 0.0)
    nc.gpsimd.memset(opad_v[:, 1:PW - 1, PW - 1:PW], 0.0)

    # ---- Phase 1: qkv conv -> qkv (SBUF, bf16, [48, v, t]) ----
    qkv = persist.tile([CH3, V, T], BF16)
    for m in range(8):
        ps = psum1.tile([IN, 512], F32, tag="mm")
        for tap in range(9):
            dy, dx = tap // 3, tap % 3
            rhs = xpad_v[:, m * 8 + dy: m * 8 + dy + 8, dx: dx + T]
            nc.tensor.matmul(
                ps[0:CH3, :], wqkv[:, tap * CH3:(tap + 1) * CH3], rhs,
                start=(tap == 0), stop=(tap == 8))
        nc.scalar.activation(
            qkv[:, m * 8:(m + 1) * 8, :].rearrange("p v t -> p (v t)"),
            ps[0:CH3, :], AF.Identity, bias=bqkv[:])
        # mirror to DRAM for the cc strip gathers (flat-layout source)
        nc.sync.dma_start(qkv_h.ap()[:, m * 8:(m + 1) * 8, :],
                          qkv[:, m * 8:(m + 1) * 8, :])

    # ---- Phase 2: spreads, all SBUF->SBUF ----
    # Branch A operands: [(c2 v), b, t] <- qkv[2b+c2, v, t]
    QA = persist.tile([128, 8, T], BF16)
    KA = persist.tile([128, 8, T], BF16)
    VA = persist.tile([128, 8, T], BF16)
    for i, dst in enumerate((QA, KA, VA)):
        src = qkv_h.ap()[i * C:(i + 1) * C]  # [16, 64, 64] (DRAM mirror)
        for c2 in range(2):
            nc.sync.dma_start(
                dst[c2 * 64:(c2 + 1) * 64, :, :],
                src.rearrange("(b c2) v t -> c2 v b t", c2=2)[c2])

    # cc operands, slot pi = a*64+b -> partition P = (a%2)*64+b, chunk ah=a//2.
    # q/k strips at spatially transposed positions: flat offset within the
    # 16-channel block = 4096*(b//4) + 1024*(b%4) + 32*ah + 16*al + j,
    # i.e. channel p=b//4, free (bl ah al j) contiguous split (4, 32, 2, 16).
    q_cc = persist.tile([128, 32, C], BF16)
    k_cc = persist.tile([128, 32, C], BF16)
    v_cc = persist.tile([128, 32, C], BF16)
    for i, dst in ((0, q_cc), (1, k_cc)):
        flat = qkv_h.ap()[i * C:(i + 1) * C].flatten()  # [65536]
        srcv = flat.rearrange("(p64 ah al j) -> al p64 ah j",
                              p64=64, al=2, ah=32, j=C)
        for al in range(2):
            nc.sync.dma_start(dst[al * 64:(al + 1) * 64, :, :], srcv[al])
    # v strips natural: offset 16*pi = 2048*ah + 1024*al + 16*b
    vflat = qkv_h.ap()[2 * C:3 * C].flatten()
    vsv = vflat.rearrange("(ah al b j) -> al b ah j", ah=32, al=2, b=64, j=C)
    for al in range(2):
        nc.sync.dma_start(v_cc[al * 64:(al + 1) * 64, :, :], vsv[al])

    # ---- Phase 3: cc compute ----
    # P on Pool, exp on ACT, H-mul + fold chains on DVE.
    S_cc = persist.tile([128, 32, C], F32)
    W_cc = persist.tile([128, 32, C], F32)
    NPH = 8
    for chk in range(32 // NPH):
        sl = slice(chk * NPH, (chk + 1) * NPH)
        qs = q_cc[:, sl, :]
        ks = k_cc[:, sl, :]
        vs = v_cc[:, sl, :]
        Pc = pP.tile([128, NPH, C, C], F32, tag="Pcc")
        nc.gpsimd.tensor_mul(
            Pc[:],
            qs.unsqueeze(3).broadcast_to([128, NPH, C, C]),
            ks.unsqueeze(2).broadcast_to([128, NPH, C, C]))
        Gc = pG.tile([128, NPH, C, C], BF16, tag="Gcc")
        nc.scalar.activation(Gc[:], Pc[:], AF.Exp)
        # S chain: fold 16->8->4, then f32 reduce over 4
        F1 = pF.tile([128, NPH, C, 8], BF16, tag="ccF1")
        nc.gpsimd.tensor_add(F1[:], Gc[:, :, :, 0:8], Gc[:, :, :, 8:16])
        F2 = pF.tile([128, NPH, C, 4], BF16, tag="ccF2")
        nc.gpsimd.tensor_add(F2[:], F1[:, :, :, 0:4], F1[:, :, :, 4:8])
        nc.vector.tensor_reduce(S_cc[:, sl, :], F2[:], axis=AX.X, op=ALU.add)
        # W chain: H = G*v, fold, reduce
        Hc = pH.tile([128, NPH, C, C], BF16, tag="Hcc")
        nc.vector.tensor_mul(
            Hc[:], Gc[:], vs.unsqueeze(2).broadcast_to([128, NPH, C, C]))
        F1w = pF.tile([128, NPH, C, 8], BF16, tag="ccF1w")
        nc.vector.tensor_add(F1w[:], Hc[:, :, :, 0:8], Hc[:, :, :, 8:16])
        F2w = pF.tile([128, NPH, C, 4], BF16, tag="ccF2w")
        nc.vector.tensor_add(F2w[:], F1w[:, :, :, 0:4], F1w[:, :, :, 4:8])
        nc.vector.tensor_reduce(W_cc[:, sl, :], F2w[:], axis=AX.X, op=ALU.add)

    # ---- Phase 4: PE back-transpose [128,16] chunks -> S_T/W_T [16, 4096] ----
    S_T = persist.tile([C, NPOS], BF16)
    W_T = persist.tile([C, NPOS], BF16)
    for dst_sb, src_cc, nm in ((S_T, S_cc, "S"), (W_T, W_cc, "W")):
        for g in range(8):  # 4 chunks per PSUM tile
            pt = psum2.tile([C, 512], F32, tag="ps2")
            for i in range(4):
                k = g * 4 + i
                nc.tensor.matmul(
                    pt[:, i * 128:(i + 1) * 128], src_cc[:, k, :], ident[:],
                    is_transpose=True)
            nc.scalar.copy(dst_sb[:, g * 512:(g + 1) * 512], pt[:])

    # combine-side spreads: S_T/W_T -> DRAM mirror -> [(c2 v), b, t]
    nc.sync.dma_start(st_h.ap()[0:C],
                      S_T[:].rearrange("c (v t) -> c v t", v=V))
    nc.sync.dma_start(st_h.ap()[C:2 * C],
                      W_T[:].rearrange("c (v t) -> c v t", v=V))
    ScA = persist.tile([128, 8, T], BF16)
    WcA = persist.tile([128, 8, T], BF16)
    for i, dst in ((0, ScA), (1, WcA)):
        src = st_h.ap()[i * C:(i + 1) * C]  # [16, 64, 64]
        for c2 in range(2):
            nc.sync.dma_start(
                dst[c2 * 64:(c2 + 1) * 64, :, :],
                src.rearrange("(b c2) v t -> c2 v b t", c2=2)[c2])

    # ---- Phase 5: branch A ----
    S_all = persist.tile([128, 8, T], F32)
    W_all = persist.tile([128, 8, T], F32)
    # transpose outputs must land at PSUM partition 0 -> one [64, 512]
    # accumulator per c2 half (4 banks total)
    S2P = [psumS.tile([64, 8 * T], F32, tag=f"S2P{c2}", name=f"S2P{c2}")
           for c2 in range(2)]
    W2P = [psumS.tile([64, 8 * T], F32, tag=f"W2P{c2}", name=f"W2P{c2}")
           for c2 in range(2)]

    for b in range(8):
        Qb = QA[:, b, :]
        Kb = KA[:, b, :]
        Vb = VA[:, b, :]

        P = pP.tile([128, T, T], F32, tag="P")
        nc.gpsimd.tensor_mul(
            P[:],
            Qb.unsqueeze(2).broadcast_to([128, T, T]),
            Kb.unsqueeze(1).broadcast_to([128, T, T]))
        G = pG.tile([128, T, T], BF16, tag="G")
        nc.scalar.activation(G[:], P[:], AF.Exp)
        Sb = S_all[:, b, :]
        Wb = W_all[:, b, :]
        # S chain: fold 64->32->16, f32 reduce over 16
        F1 = pF.tile([128, T, 32], BF16, tag="F1")
        nc.gpsimd.tensor_add(F1[:], G[:, :, 0:32], G[:, :, 32:64])
        F2 = pF.tile([128, T, C], BF16, tag="F2")
        nc.gpsimd.tensor_add(F2[:], F1[:, :, 0:16], F1[:, :, 16:32])
        nc.vector.tensor_reduce(Sb, F2[:], axis=AX.X, op=ALU.add)
        # W chain
        H = pH.tile([128, T, T], BF16, tag="H")
        nc.vector.tensor_mul(
            H[:], G[:], Vb.unsqueeze(1).broadcast_to([128, T, T]))
        F1w = pF.tile([128, T, 32], BF16, tag="F1w")
        nc.vector.tensor_add(F1w[:], H[:, :, 0:32], H[:, :, 32:64])
        F2w = pF.tile([128, T, C], BF16, tag="F2w")
        nc.vector.tensor_add(F2w[:], F1w[:, :, 0:16], F1w[:, :, 16:32])
        nc.vector.tensor_reduce(Wb, F2w[:], axis=AX.X, op=ALU.add)

        # vv terms via PE transposes into the batched PSUM accumulators
        for c2 in range(2):
            rows = slice(c2 * 64, (c2 + 1) * 64)
            cols = slice(b * T, (b + 1) * T)
            nc.tensor.matmul(S2P[c2][:, cols], Sb[rows], ident[rows, rows],
                             is_transpose=True)
            nc.tensor.matmul(W2P[c2][:, cols], Wb[rows], ident[rows, rows],
                             is_transpose=True)

    # ---- Phase 5b: batched combine ----
    Sf = S_all[:].rearrange("p b t -> p (b t)")
    Wf = W_all[:].rearrange("p b t -> p (b t)")
    ScAf = ScA[:].rearrange("p b t -> p (b t)")
    WcAf = WcA[:].rearrange("p b t -> p (b t)")
    Z = pcomb.tile([128, 8 * T], F32, tag="Z")
    for c2 in range(2):
        rows = slice(c2 * 64, (c2 + 1) * 64)
        nc.vector.tensor_add(Z[rows, :], Sf[rows], S2P[c2][:])
    nc.gpsimd.tensor_add(Z[:], Z[:], ScAf)
    R = pcomb.tile([128, 8 * T], F32, tag="R")
    nc.vector.reciprocal(R[:], Z[:])
    Ot = pcomb.tile([128, 8 * T], BF16, tag="Ot")
    nc.vector.tensor_mul(Ot[:], Wf, R[:])
    Ov = pcomb.tile([128, 8 * T], BF16, tag="Ov")
    for c2 in range(2):
        rows = slice(c2 * 64, (c2 + 1) * 64)
        nc.vector.tensor_mul(Ov[rows, :], W2P[c2][:], R[rows, :])
    Oc = pcomb.tile([128, 8 * T], BF16, tag="Oc")
    nc.gpsimd.tensor_mul(Oc[:], WcAf, R[:])

    # ---- Phase 5c: outputs -> channel-major DRAM -> opad (one wide gather) -
    for g, t_ in ((0, Oc), (1, Ov), (2, Ot)):
        tv = t_[:].rearrange("p (b t) -> p b t", b=8)
        dstv = ocat_h.ap()[g * C:(g + 1) * C].rearrange(
            "(b c2) v t -> c2 v b t", c2=2)
        for c2 in range(2):
            nc.sync.dma_start(dstv[c2], tv[c2 * 64:(c2 + 1) * 64])
    nc.sync.dma_start(opad_v[:, 1:1 + V, 1:1 + T], ocat_h.ap())

    # ---- Phase 6: reverse conv + residual ----
    for m in range(8):
        ps2 = psum1.tile([IN, 512], F32, tag="mm")
        for tap in range(9):
            dy, dx = tap // 3, tap % 3
            rhs = opad_v[:, m * 8 + dy: m * 8 + dy + 8, dx: dx + T]
            nc.tensor.matmul(
                ps2[:], wr[:, tap * IN:(tap + 1) * IN], rhs,
                start=(tap == 0), stop=(tap == 8))
        o_sb = pio.tile([IN, 512], F32, tag="o_sb")
        xin = xpad_v[:, m * 8 + 1: m * 8 + 9, 1:1 + T]
        nc.vector.scalar_tensor_tensor(
            o_sb[:].rearrange("p (v t) -> p v t", v=8),
            ps2[:].rearrange("p (v t) -> p v t", v=8),
            gb[:], xin, op0=ALU.add, op1=ALU.add)
        nc.sync.dma_start(out_d.ap()[:, m * 8:(m + 1) * 8, :],
                          o_sb[:].rearrange("p (v t) -> p v t", v=8))

    ctx.close()


_NC_CACHE = {}


def _get_program(niter=1):
    if niter not in _NC_CACHE:
        _NC_CACHE[niter] = _build_program(niter)
    return _NC_CACHE[niter]


def _host_weights(wq, bq, wk, bk, wv, bv, wcr, bcr, wvr, bvr, wtr, btr, gamma):
    g = np.float32(np.asarray(gamma).reshape(-1)[0])
    wf = np.concatenate([wq, wk, wv], axis=0)  # [48, 128, 3, 3]
    wqkv = np.ascontiguousarray(
        wf.transpose(1, 2, 3, 0).reshape(IN, 9 * CH3)).astype(ml_dtypes.bfloat16)
    bqkv = np.concatenate([bq, bk, bv]).reshape(CH3, 1).astype(np.float32)
    wrf = np.concatenate([wcr, wvr, wtr], axis=1) * g  # [128, 48, 3, 3]
    wr_ = np.ascontiguousarray(
        wrf.transpose(1, 2, 3, 0).reshape(CH3, 9 * IN)).astype(ml_dtypes.bfloat16)
    gb = (g * (bcr + bvr + btr)).reshape(IN, 1).astype(np.float32)
    return wqkv, bqkv, wr_, gb


def kernel(x, wq, bq, wk, bk, wv, bv, wcr, bcr, wvr, bvr, wtr, btr, gamma,
           _trace=False, _niter=1):
    nc = _get_program(_niter)
    wqkv, bqkv, wr_, gb = _host_weights(
        wq, bq, wk, bk, wv, bv, wcr, bcr, wvr, bvr, wtr, btr, gamma)
    x = np.asarray(x, dtype=np.float32).astype(ml_dtypes.bfloat16)
    ident = np.eye(128, dtype=np.float32)
    in_maps = [
        {"x": np.ascontiguousarray(x[i]), "wqkv": wqkv, "bqkv": bqkv,
         "wr": wr_, "gb": gb, "ident": ident}
        for i in range(8)
    ]
    res = run_bass_kernel_spmd(nc, in_maps, list(range(8)), trace=_trace)
    out = np.stack([res.results[i]["out"] for i in range(8)]).astype(np.float32)
    if _trace:
        kernel.last_exec_time_ns = res.exec_time_ns
        kernel.last_results = res
    return out
